# revision 15
# baseline (speedup 1.0000x reference)
"""MultiHeadSelfAttention Trainium2 kernel, 8-way sharded.

Sharding (Megatron-style, per spec hint): 2 batch groups x 4 tensor-parallel
head groups. Core c handles batch c//4 and heads 4*(c%4)..4*(c%4)+3, i.e. a
256-column slice of Wq/Wk/Wv and the matching 256-row slice of Wo. Each core
returns a partial out^T [1024, 2048]; the host sums the 4 TP partials per
batch and adds bo.

Per-core pipeline (S=2048 tokens, 4 heads, dk=64):
  x -> (PE transpose) X^T -> Q^T,K^T (f32r matmuls, f16 stored)
                            V (natural layout, f16, with ones column)
  per (tq-half, head-pair): S^T = K^T.T @ Q^T  (f16, tile_position-packed)
                            P^T = exp(S^T/8)   (ACT, psum->sbuf, f16)
                            O^T' = [V|1].T @ P^T accumulated over tk (psum)
                            row 64 of O^T' = softmax denominator
                            normalize via DVE recip + K=1 broadcast matmul
  out^T = Wo_slice.T-layout matmul over AttnOut^T (f32r), DMA to DRAM.
"""
import numpy as np

import concourse.bass as bass
import concourse.mybir as mybir
import concourse.tile as tile
from concourse.masks import make_identity
from concourse.vector_clock import ScopedClock

# ---------------------------------------------------------------------------
# Workaround: this container's walrus only encodes 1 sync-wait per
# instruction, but TileContext's final drain carries one wait per logical
# proc. Split extra waits onto standalone drains (same SP engine, sequential,
# semantically identical).
def _patched_drain_and_barrier(self, tick_clock, wait_clock):
    nc = self.nc
    drain_inst = nc.sync.drain()
    wait_clock.add_sem_waits(
        drain_inst.ins, ScopedClock({None: tick_clock.global_clock})
    )
    si = drain_inst.ins.sync_info
    waits = list(si.on_wait or [])
    if len(waits) > 1:
        si.on_wait = waits[:1]
        for w in waits[1:]:
            extra = nc.sync.drain()
            extra.ins.sync_info = mybir.SyncInfo(on_wait=[w], on_update=[])
    nc.all_engine_barrier()
    assert self.sems is not None
    popped = nc._tile_sem_poison_stack.pop()
    assert popped is self._sem_poison
    nc.clear_and_free_semaphores(list(self.sems.allocated().values()))
    nc.all_engine_barrier()


tile.TileContext._drain_and_barrier = _patched_drain_and_barrier


def split_multi_waits(nc):
    """Walrus in this container encodes at most one sync-wait per
    instruction. Hoist extra waits onto injected same-engine NoOps placed
    immediately before the instruction (same engine executes in order, so
    semantics are identical)."""
    uid = 0
    for f in nc.m.functions:
        for bb in f.blocks:
            new = []
            changed = False
            for inst in bb.instructions:
                si = inst.sync_info
                waits = list(si.on_wait or []) if si is not None else []
                if len(waits) > 1:
                    changed = True
                    for w in waits[:-1]:
                        uid += 1
                        n = mybir.InstNoOp(
                            name=f"waitsplit_{uid}", ins=[], outs=[]
                        )
                        n.engine = inst.engine
                        n.sync_info = mybir.SyncInfo(on_wait=[w], on_update=[])
                        new.append(n)
                    si.on_wait = waits[-1:]
                new.append(inst)
            if changed:
                bb.instructions = new
# ---------------------------------------------------------------------------

FP32 = mybir.dt.float32
F32R = mybir.dt.float32r
F16 = mybir.dt.float16

D_MODEL = 1024
S = 2048
DK = 64
H_PER_CORE = 4          # heads per core
DC = H_PER_CORE * DK    # 256 d_model columns per core
N_CORES = 8
TP = 4                  # tensor-parallel groups
B = 2

KT = D_MODEL // 128     # 8 k-tiles of X^T
TT = S // 128           # 16 token tiles
TQ5 = S // 512          # 4 tq 512-groups


def r32(ap):
    return ap.bitcast(F32R)


def build_nc():
    nc = bass.Bass("TRN2")

    x = nc.dram_tensor("x", [S, D_MODEL], FP32, kind="ExternalInput")
    wq = nc.dram_tensor("wq", [D_MODEL, DC], FP32, kind="ExternalInput")
    wk = nc.dram_tensor("wk", [D_MODEL, DC], FP32, kind="ExternalInput")
    wv = nc.dram_tensor("wv", [D_MODEL, DC], FP32, kind="ExternalInput")
    wo = nc.dram_tensor("wo", [DC, D_MODEL], FP32, kind="ExternalInput")
    bq = nc.dram_tensor("bq", [DC], FP32, kind="ExternalInput")
    bk = nc.dram_tensor("bk", [DC], FP32, kind="ExternalInput")
    bv = nc.dram_tensor("bv", [DC], FP32, kind="ExternalInput")
    out = nc.dram_tensor("out", [D_MODEL, S], FP32, kind="ExternalOutput")

    with tile.TileContext(nc) as tc:
        with (
            tc.tile_pool(name="consts", bufs=1) as consts,
            tc.tile_pool(name="weights", bufs=1) as wpool,
            tc.tile_pool(name="acts", bufs=1) as acts,
            tc.tile_pool(name="xnat", bufs=2) as xnat_pool,
            tc.tile_pool(name="xtc", bufs=2) as xtc_pool,
            tc.tile_pool(name="pt", bufs=1) as pt_pool,
            tc.tile_pool(name="small", bufs=2) as small,
            tc.tile_pool(name="stage", bufs=4) as stage_pool,
            tc.tile_pool(name="dram", bufs=2, space="DRAM") as dram_pool,
            tc.tile_pool(name="ps", bufs=2, space="PSUM") as ps_pool,
            tc.tile_pool(name="pso", bufs=2, space="PSUM") as pso_pool,
        ):
            # ---------------- constants ----------------
            ident = consts.tile([128, 128], FP32)
            make_identity(nc, ident)
            ones = consts.tile([1, 512], F32R)
            nc.vector.memset(ones.bitcast(FP32), 1.0)

            bq_sb = consts.tile([1, DC], F32R)
            bk_sb = consts.tile([1, DC], F32R)
            bv_sb = consts.tile([1, DC], F32R)
            nc.sync.dma_start(out=bq_sb, in_=bq.rearrange("(a n) -> a n", a=1).bitcast(F32R))
            nc.sync.dma_start(out=bk_sb, in_=bk.rearrange("(a n) -> a n", a=1).bitcast(F32R))
            nc.sync.dma_start(out=bv_sb, in_=bv.rearrange("(a n) -> a n", a=1).bitcast(F32R))

            # ---------------- weights ----------------
            wq_sb = wpool.tile([128, KT, DC], F32R)
            wk_sb = wpool.tile([128, KT, DC], F32R)
            wv_sb = wpool.tile([128, KT, DC], F32R)
            wo_sb = wpool.tile([128, 2, D_MODEL], F32R)
            nc.sync.dma_start(out=wq_sb, in_=wq.rearrange("(kt p) n -> p kt n", p=128).bitcast(F32R))
            nc.sync.dma_start(out=wk_sb, in_=wk.rearrange("(kt p) n -> p kt n", p=128).bitcast(F32R))
            nc.sync.dma_start(out=wv_sb, in_=wv.rearrange("(kt p) n -> p kt n", p=128).bitcast(F32R))
            nc.sync.dma_start(out=wo_sb, in_=wo.rearrange("(c p) n -> p c n", p=128).bitcast(F32R))

            # ---------------- activations (persistent) ----------------
            # Q^T/K^T: [d-in-chunk, dq-chunk(2 heads each), tokens], f16
            qT = acts.tile([128, 2, S], F16)
            kT = acts.tile([128, 2, S], F16)
            # V with ones column: [t-in-tile, t-tile, head, 65], f16
            v65 = acts.tile([128, TT, H_PER_CORE, 65], F16)
            nc.vector.memset(v65[:, :, :, 64:65], 1.0)
            # AttnOut^T: [dh-in-chunk, dh-chunk, tokens], f32
            attnT = acts.tile([128, 2, S], F32R)

            # =============== phase 1: projections ===============
            for tg in range(TQ5):  # token groups of 512
                xtc = xtc_pool.tile([128, KT, 512], F32R, tag="xtc")
                for i in range(4):
                    tt = tg * 4 + i
                    xn = xnat_pool.tile([128, D_MODEL], FP32, tag="xn")
                    nc.sync.dma_start(out=xn, in_=x[tt * 128:(tt + 1) * 128, :])
                    for half in range(2):
                        tp = ps_pool.tile([128, 1024], FP32, tag="ps")
                        for j in range(4):
                            kt = half * 4 + j
                            # transpose: out = xn-block^T
                            nc.tensor.transpose(
                                tp[:, j * 128:(j + 1) * 128],
                                xn[:, kt * 128:(kt + 1) * 128],
                                ident,
                            )
                        nc.vector.tensor_copy(
                            xtc[:, half * 4:half * 4 + 4, i * 128:(i + 1) * 128],
                            tp[:, 0:512].rearrange("p (a b) -> p a b", a=4),
                        )

                # Q^T and K^T for this token group
                ts512 = slice(tg * 512, (tg + 1) * 512)
                for w_sb, b_sb, dst in ((wq_sb, bq_sb, qT), (wk_sb, bk_sb, kT)):
                    for dqc in range(2):
                        pq = ps_pool.tile([128, 1024], FP32, tag="ps")
                        for kt in range(KT):
                            nc.tensor.matmul(
                                pq[:, 0:512],
                                r32(w_sb[:, kt, dqc * 128:(dqc + 1) * 128]),
                                r32(xtc[:, kt, :]),
                                start=(kt == 0), stop=False,
                            )
                        nc.tensor.matmul(
                            pq[:, 0:512],
                            r32(b_sb[0:1, dqc * 128:(dqc + 1) * 128]),
                            r32(ones[0:1, :]),
                            start=False, stop=True,
                        )
                        nc.vector.tensor_copy(dst[:, dqc, ts512], pq[:, 0:512])

                # V for the 4 token tiles of this group
                for i in range(4):
                    tt = tg * 4 + i
                    pv = ps_pool.tile([128, 1024], FP32, tag="ps")
                    for kt in range(KT):
                        nc.tensor.matmul(
                            pv[:, 0:DC],
                            r32(xtc[:, kt, i * 128:(i + 1) * 128]),
                            r32(wv_sb[:, kt, :]),
                            start=(kt == 0), stop=False,
                        )
                    nc.tensor.matmul(
                        pv[:, 0:DC],
                        r32(ones[0:1, 0:128]),
                        r32(bv_sb[0:1, :]),
                        start=False, stop=True,
                    )
                    nc.vector.tensor_copy(
                        v65[:, tt, :, 0:64],
                        pv[:, 0:DC].rearrange("p (h d) -> p h d", h=H_PER_CORE),
                    )

            # =============== phase 2: attention ===============
            for tqh in range(2):          # halves of the query axis
                for pair in range(2):     # head pairs (= dq chunk)
                    pt = pt_pool.tile([128, TT, 2, 2, 512], F16, tag="pt")
                    o_ps = [
                        pso_pool.tile([65, 1024], FP32, tag="o", name=f"o_ps{_h}")
                        for _h in range(2)
                    ]
                    for tk in range(TT):
                        for tqc in range(2):
                            tq0 = tqh * 1024 + tqc * 512
                            sc = ps_pool.tile([128, 1024], FP32, tag="ps")
                            for hh in range(2):  # head within pair
                                po = hh * 64
                                nc.tensor.matmul(
                                    sc[:, hh * 512:(hh + 1) * 512],
                                    kT[po:po + 64, pair, tk * 128:(tk + 1) * 128],
                                    qT[po:po + 64, pair, tq0:tq0 + 512],
                                )
                            nc.scalar.activation(
                                pt[:, tk, :, tqc, :],
                                sc.rearrange("p (h n) -> p h n", h=2),
                                mybir.ActivationFunctionType.Exp,
                                scale=0.125,
                            )
                            for hh in range(2):
                                nc.tensor.matmul(
                                    o_ps[hh][0:65, tqc * 512:(tqc + 1) * 512],
                                    v65[:, tk, pair * 2 + hh, :],
                                    pt[:, tk, hh, tqc, :],
                                    start=(tk == 0), stop=(tk == TT - 1),
                                )
                    # normalize each head of the pair
                    for hh in range(2):
                        r_sb = small.tile([1, 1024], FP32, tag="r")
                        nc.vector.reciprocal(r_sb, o_ps[hh][64:65, :])
                        # broadcast along partitions via stride-0 DMA
                        rb_sb = small.tile([64, 1024], FP32, tag="rb")
                        r_dram = dram_pool.tile([1, 1024], FP32, tag="rd")
                        nc.sync.dma_start(out=r_dram, in_=r_sb)
                        r_bc = bass.AP(
                            tensor=r_dram.tensor,
                            offset=r_dram.offset,
                            ap=[[0, 64]] + [list(a) for a in r_dram.ap[1:]],
                        )
                        nc.sync.dma_start(out=rb_sb, in_=r_bc)
                        po = hh * 64
                        nc.vector.tensor_mul(
                            attnT[po:po + 64, pair, tqh * 1024:(tqh + 1) * 1024],
                            o_ps[hh][0:64, :],
                            rb_sb,
                        )

                # =============== Wo for this half ===============
                for do in range(8):
                    for tqc in range(2):
                        tq0 = tqh * 1024 + tqc * 512
                        pw = ps_pool.tile([128, 1024], FP32, tag="ps")
                        for dhc in range(2):
                            nc.tensor.matmul(
                                pw[:, 0:512],
                                r32(wo_sb[:, dhc, do * 128:(do + 1) * 128]),
                                r32(attnT[:, dhc, tq0:tq0 + 512]),
                                start=(dhc == 0), stop=(dhc == 1),
                            )
                        st = stage_pool.tile([128, 512], FP32, tag="st")
                        nc.vector.tensor_copy(st, pw[:, 0:512])
                        nc.sync.dma_start(
                            out=out[do * 128:(do + 1) * 128, tq0:tq0 + 512],
                            in_=st,
                        )
    split_multi_waits(nc)
    return nc


_NC_CACHE = None


def _get_nc():
    global _NC_CACHE
    if _NC_CACHE is None:
        _NC_CACHE = build_nc()
    return _NC_CACHE


def make_in_maps(inputs):
    x = np.ascontiguousarray(np.asarray(inputs["x"], dtype=np.float32))
    maps = []
    for c in range(N_CORES):
        b, g = c // TP, c % TP
        cs = slice(g * DC, (g + 1) * DC)
        maps.append({
            "x": np.ascontiguousarray(x[b]),
            "wq": np.ascontiguousarray(np.asarray(inputs["Wq"], np.float32)[:, cs]),
            "wk": np.ascontiguousarray(np.asarray(inputs["Wk"], np.float32)[:, cs]),
            "wv": np.ascontiguousarray(np.asarray(inputs["Wv"], np.float32)[:, cs]),
            "wo": np.ascontiguousarray(np.asarray(inputs["Wo"], np.float32)[cs, :]),
            "bq": np.ascontiguousarray(np.asarray(inputs["bq"], np.float32)[cs]),
            "bk": np.ascontiguousarray(np.asarray(inputs["bk"], np.float32)[cs]),
            "bv": np.ascontiguousarray(np.asarray(inputs["bv"], np.float32)[cs]),
        })
    return maps


def assemble(results, bo):
    out = np.zeros((B, S, D_MODEL), dtype=np.float32)
    for c in range(N_CORES):
        b = c // TP
        out[b] += results[c]["out"].T
    out += np.asarray(bo, np.float32)[None, None, :]
    return out


def kernel(**inputs):
    from concourse.bass_utils import run_bass_kernel_spmd

    nc = _get_nc()
    res = run_bass_kernel_spmd(nc, make_in_maps(inputs), core_ids=list(range(N_CORES)))
    return assemble(res.results, inputs["bo"])
